# revision 1
# baseline (speedup 1.0000x reference)
"""Causal multi-head attention (B=2, T=2048, C=1024, H=16, D=64) on 8 trn2 cores.

Sharding: core c -> (batch c//4, head-group c%4 of 4 heads / 256 channels).
Each core computes q/k/v for its head group, causal attention, and a partial
output projection y_part[2048,1024] = attnout_g @ wo_g.T. The host sums the 4
per-group partials of each batch (the "all-reduce after wo" done host-side).

Device kernel (per core, SPMD identical program):
  phase B: xT,[wq|wk|wv]T loaded + rounded to float32r; q^T,k^T (head-channel
           major) and v (natural, with a ones column -> vaug) via PE matmuls.
  phase C: per head, per tq-tile(512): ST[tk128,tq512] = k^T.T @ q^T (K=64),
           P = exp(ST/8) on ScalarE (PSUM->SBUF, float32r), diagonal blocks
           multiplied by 0/1 masks, PV accumulated over tk into PSUM[65,512]
           where row 64 (ones column of vaug) is the softmax denominator;
           normalize with reciprocal + K=1 broadcast matmul.
  phase D: y[t,1024] = attnoutT.T @ woT, DMA'd out per 128-row tile.

All matmuls run in float32r (TF32-like, full PE rate at N>=256).
"""
import time
import hashlib
import numpy as np

import jax
import jax.numpy as jnp
from jax.sharding import Mesh, PartitionSpec
from jax.experimental.shard_map import shard_map

import concourse.bass as bass
import concourse.tile as tile
from concourse import bacc, mybir
from concourse import bass2jax
from concourse.bass2jax import _bass_exec_p, install_neuronx_cc_hook, partition_id_tensor

B, T, C = 2, 2048, 1024
H = 16
D = C // H            # 64
SCALE = D ** -0.5     # 0.125
N_CORES = 8
HG = H // (N_CORES // B)   # heads per core = 4
HC = HG * D                # channels per core = 256
KT = C // 128              # 8 contraction tiles
NT = T // 128              # 16 row tiles
NJ = T // 512              # 4 tq tiles
F32 = mybir.dt.float32
F32R = mybir.dt.float32r


# ---------------------------------------------------------------- device code

def _build_nc(block_info, n_uniq, loop_n=None, phases="BCD", cast_dma=False):
    """block_info[j][i] = None (skip) | -1 (full) | idx>=0 (mask tile index).

    Inputs are declared float32r: the host pre-rounds to 11 mantissa bits
    (RNE), so plain HWDGE DMA loads land PE-ready with no casting pass.
    """
    nc = bacc.Bacc("TRN2", target_bir_lowering=False, debug=False,
                   num_devices=N_CORES)
    xT_ap = nc.dram_tensor("xT", [C, T], F32R, kind="ExternalInput").ap()
    # weights host-packed so each SBUF partition's bytes are contiguous in
    # DRAM (8KB descriptors; scattered 1KB descriptors measured ~4x slower)
    wqT_ap = nc.dram_tensor("wqT", [128, KT * HC], F32R, kind="ExternalInput").ap()
    wkT_ap = nc.dram_tensor("wkT", [128, KT * HC], F32R, kind="ExternalInput").ap()
    wvT_ap = nc.dram_tensor("wvT", [128, KT * HC], F32R, kind="ExternalInput").ap()
    woT_ap = nc.dram_tensor("woT", [128, 2 * C], F32R, kind="ExternalInput").ap()
    mk_ap = nc.dram_tensor("mk", [128, max(n_uniq, 1) * 512], F32,
                           kind="ExternalInput").ap()
    y_ap = nc.dram_tensor("y", [T, C], F32, kind="ExternalOutput").ap()

    with tile.TileContext(nc) as tc:
        with (
            nc.allow_low_precision(reason="float32r (tf32-like) matmul pipeline"),
            tc.tile_pool(name="glob", bufs=1) as pg,
            tc.tile_pool(name="warm", bufs=1) as pwarm,
        ):
            # persistent across phases
            qT = pg.tile([128, 2, T], F32R)        # [o-part, o-tile, t]
            # k^T zero-padded per head: head h lives in partition rows
            # 64*(h%2)..+64 of kTz[:, h, :], other rows stay 0 so the score
            # matmul runs at K=128 (fp32r is slow for K<128).
            kTz = pg.tile([128, HG, T], F32R)
            # v natural per tk-tile/head, padded to 128 cols: [v | 1 | zeros]
            # (fp32r is slow for M<128; ones column gives softmax denom).
            vaug = pg.tile([128, NT, HG, 128], F32R)
            ones128 = pg.tile([128, 128], F32R)    # all-ones lhsT for bcast
            recipz = pg.tile([128, 512], F32R)     # row0=recip, rows1-127 zero
            ident = pg.tile([128, 128], F32)       # PE-transpose identity

            # warm the Exp table while DMAs run; f32r rejects memset, so
            # zeros/ones are produced via rounding copies from f32 tiles.
            wtile = pwarm.tile([1, 16], F32)
            nc.vector.memset(wtile[:], 0.0)
            nc.scalar.activation(wtile[:], wtile[:],
                                 mybir.ActivationFunctionType.Exp)
            zt = pwarm.tile([128, 512], F32)
            nc.vector.memset(zt[:], 0.0)
            nc.vector.tensor_copy(kTz[:].rearrange("p h (j f) -> p h j f", f=512), zt[:, None, None, :].broadcast_to([128, HG, NJ, 512]))
            nc.vector.tensor_copy(vaug[:], zt[:, None, None, 0:128].broadcast_to([128, NT, HG, 128]))
            nc.vector.tensor_copy(recipz[:], zt[:])
            import concourse.masks as _masks
            _masks.make_identity(nc, ident[:])
            ot = pwarm.tile([128, 16], F32)
            nc.vector.memset(ot[:], 1.0)
            nc.vector.tensor_copy(vaug[:, :, :, D], ot[:, None, 0:HG].broadcast_to([128, NT, HG]))
            nc.vector.tensor_copy(ones128[:], ot[:, 0:1].broadcast_to([128, 128]))

            def body():
                # ---------------- phase A/B: loads + projections ------------
                with (
                    tc.tile_pool(name="ab", bufs=1) as pab,
                    tc.tile_pool(name="wp", bufs=2) as pwp,
                    tc.tile_pool(name="psq", bufs=4, space="PSUM") as psq,
                    tc.tile_pool(name="psv", bufs=3, space="PSUM") as psv,
                ):
                    xT = pab.tile([128, KT, T], F32R)
                    vT = pab.tile([128, 2, T], F32R)
                    # chunked so projections start as soon as chunk 0 lands
                    for kc in range(KT):
                        nc.sync.dma_start(
                            xT[:, kc, :],
                            xT_ap.rearrange("(k p) t -> k p t", p=128)[kc])
                    if "B" not in phases:
                        return

                    # q^T, k^T, v^T: [o, t] = w_g @ x^T ; kc-outer
                    for w_ap, qk in ((wqT_ap, 0), (wkT_ap, 1), (wvT_ap, 2)):
                        w_t = pwp.tile([128, KT, HC], F32R, tag="w", name=f"w{qk}")
                        nc.sync.dma_start(w_t[:], w_ap.rearrange("p (k m) -> p k m", k=KT))
                        for m in range(2):
                            pss = [psq.tile([128, 512], F32, tag="qkps", name=f"qkps{m}_{j}") for j in range(NJ)]
                            for kc in range(KT):
                                for j in range(NJ):
                                    nc.tensor.matmul(
                                        pss[j][:],
                                        w_t[:, kc, 128 * m:128 * (m + 1)],
                                        xT[:, kc, 512 * j:512 * (j + 1)],
                                        start=(kc == 0), stop=(kc == KT - 1))
                            for j in range(NJ):
                                sl = slice(512 * j, 512 * (j + 1))
                                if qk == 0:
                                    nc.scalar.copy(qT[:, m, sl], pss[j][:])
                                elif qk == 2:
                                    nc.scalar.copy(vT[:, m, sl], pss[j][:])
                                else:
                                    # scatter psum head-halves into kTz rows
                                    nc.scalar.copy(kTz[0:64, 2 * m, sl],
                                                   pss[j][0:64, :])
                                    nc.scalar.copy(kTz[64:128, 2 * m + 1, sl],
                                                   pss[j][64:128, :])
                    # v natural via PE transpose of vT 128x128 blocks
                    for m in range(2):
                        for i in range(NT):
                            ps = psv.tile([128, 128], F32, tag="vtp", name=f"vtp{m}_{i}")
                            nc.tensor.transpose(
                                ps[:], vT[:, m, 128 * i:128 * (i + 1)].bitcast(F32),
                                ident[:])
                            nc.vector.tensor_copy(
                                vaug[:, i, 2 * m:2 * m + 2, 0:D],
                                ps[:].rearrange("p (h d) -> p h d", h=2))

                # -------- phase C+D: attention, interleaved with out-proj ----
                if "C" not in phases and "D" not in phases:
                    return
                with (
                    tc.tile_pool(name="cd", bufs=1) as pcd,
                    tc.tile_pool(name="pt", bufs=4) as ppt,
                    tc.tile_pool(name="small", bufs=4) as psm,
                    tc.tile_pool(name="ys", bufs=3) as pys,
                    tc.tile_pool(name="psst", bufs=2, space="PSUM") as psst,
                    tc.tile_pool(name="pspv", bufs=2, space="PSUM") as pspv,
                    tc.tile_pool(name="psy", bufs=2, space="PSUM") as psy,
                ):
                    if n_uniq > 0:
                        mks = pcd.tile([128, max(n_uniq, 1), 512], F32)
                        nc.sync.dma_start(mks[:], mk_ap.rearrange("p (u f) -> p u f", f=512))
                    woT = pcd.tile([128, 2, C], F32R)
                    nc.sync.dma_start(woT[:], woT_ap.rearrange("p (k m) -> p k m", k=2))
                    attnoutT = pcd.tile([128, 2, T], F32R)

                    for j in range(NJ):
                        blocks = [(i, bi) for i, bi in enumerate(block_info[j])
                                  if bi is not None]
                        chunks = [blocks[c:c + 2] for c in range(0, len(blocks), 2)]
                        for h in range(HG if "C" in phases else 0):
                            m = h // 2
                            jsl = slice(512 * j, 512 * (j + 1))
                            pv = pspv.tile([128, 512], F32, tag="pv", name=f"pv{h}_{j}")
                            n_acc = len(blocks)
                            acc = 0
                            prev_chunk = None  # (pt, idxs)

                            def emit_pv(pt, idxs):
                                nonlocal acc
                                for c, i in enumerate(idxs):
                                    nc.tensor.matmul(
                                        pv[:], vaug[:, i, h, :], pt[:, c, :],
                                        start=(acc == 0), stop=(acc == n_acc - 1))
                                    acc += 1

                            for ch in chunks:
                                nsub = len(ch)
                                st = psst.tile([128, 2, 512], F32, tag="st", name=f"st{h}_{j}")
                                for c, (i, bi) in enumerate(ch):
                                    nc.tensor.matmul(
                                        st[:, c, :],
                                        kTz[:, h, 128 * i:128 * (i + 1)],
                                        qT[:, m, jsl],
                                        start=True, stop=True)
                                pt = ppt.tile([128, 2, 512], F32R, tag="pt")
                                # one exp per chunk: the ~0.9us fixed per-op
                                # ACT cost dominates, so amortize over 1024
                                nc.scalar.activation(
                                    pt[:, 0:nsub, :], st[:, 0:nsub, :],
                                    mybir.ActivationFunctionType.Exp, scale=SCALE)
                                mi = [bi for _, bi in ch]
                                if any(b >= 0 for b in mi):
                                    if nsub == 2 and mi[0] >= 0 and mi[1] == mi[0] + 1:
                                        nc.vector.tensor_mul(
                                            pt[:], pt[:], mks[:, mi[0]:mi[0] + 2, :])
                                    else:
                                        for c, b in enumerate(mi):
                                            if b >= 0:
                                                nc.vector.tensor_mul(
                                                    pt[:, c, :], pt[:, c, :],
                                                    mks[:, b, :])
                                if prev_chunk is not None:
                                    emit_pv(*prev_chunk)
                                prev_chunk = (pt, [i for i, _ in ch])
                            emit_pv(*prev_chunk)
                            # normalization: recip of denom row, bcast via PE
                            recip = psm.tile([1, 512], F32R, tag="recip")
                            nc.vector.reciprocal(recip[:], pv[64:65, :])
                            nc.vector.tensor_copy(recipz[0:1, :], recip[:])
                            bc = psy.tile([128, 512], F32, tag="yps", name=f"bc{h}_{j}")
                            nc.tensor.matmul(bc[:], ones128[:], recipz[:],
                                             start=True, stop=True)
                            avu = psm.tile([64, 512], F32, tag="avu")
                            nc.vector.tensor_copy(avu[:], pv[0:64, :])
                            row = 64 * (h % 2)
                            nc.vector.tensor_mul(
                                attnoutT[row:row + 64, m, jsl],
                                avu[:], bc[0:64, :])

                        # ---- phase D for this j: y rows [512j, 512j+512) ----
                        if "D" not in phases:
                            continue
                        for tp in range(2):     # pairs of row tiles
                            ys = pys.tile([128, 2, C], F32, tag="ys")
                            for tsub in range(2):
                                t = 4 * j + 2 * tp + tsub
                                for o2 in range(2):
                                    ps = psy.tile([128, 512], F32, tag="yps", name=f"yps{t}_{o2}")
                                    for kc in range(2):
                                        nc.tensor.matmul(
                                            ps[:],
                                            attnoutT[:, kc, 128 * t:128 * (t + 1)],
                                            woT[:, kc, 512 * o2:512 * (o2 + 1)],
                                            start=(kc == 0), stop=(kc == 1))
                                    nc.vector.tensor_copy(
                                        ys[:, tsub, 512 * o2:512 * (o2 + 1)], ps[:])
                            r0 = 512 * j + 256 * tp
                            nc.scalar.dma_start(
                                y_ap[r0:r0 + 256, :].rearrange("(tt p) o -> p tt o", p=128),
                                ys[:])

            if loop_n is None:
                body()
            else:
                with tc.For_i(0, loop_n, 1):
                    body()

    nc.compile()
    return nc


# ---------------------------------------------------------------- run harness

def _install_verbose_hook():
    install_neuronx_cc_hook()
    try:
        import libneuronxla
    except ImportError:
        return
    import traceback
    inner = bass2jax.neuronx_cc_hook

    def wrapped(*a, **kw):
        try:
            return inner(*a, **kw)
        except BaseException:
            traceback.print_exc()
            raise
    libneuronxla.neuronx_cc = wrapped


class _SpmdRunner:
    def __init__(self, nc, n_cores):
        _install_verbose_hook()
        self.nc, self.n_cores = nc, n_cores
        pname = nc.partition_id_tensor.name if nc.partition_id_tensor else None
        in_names, out_names, out_avals = [], [], []
        for alloc in nc.m.functions[0].allocations:
            if not isinstance(alloc, mybir.MemoryLocationSet):
                continue
            name = alloc.memorylocations[0].name
            if alloc.kind == "ExternalInput":
                if name != pname:
                    in_names.append(name)
            elif alloc.kind == "ExternalOutput":
                out_names.append(name)
                out_avals.append(jax.core.ShapedArray(
                    tuple(alloc.tensor_shape), mybir.dt.np(alloc.dtype)))
        self.in_names, self.out_names, self.out_avals = in_names, out_names, out_avals
        n_params = len(in_names)
        all_in = list(in_names) + list(out_names)
        if pname is not None:
            all_in.append(pname)

        def _body(*args):
            operands = list(args)
            if pname is not None:
                operands.append(partition_id_tensor())
            return tuple(_bass_exec_p.bind(
                *operands,
                out_avals=tuple(out_avals), in_names=tuple(all_in),
                out_names=tuple(out_names), lowering_input_output_aliases=(),
                sim_require_finite=True, sim_require_nnan=True, nc=nc))

        devices = jax.devices()[:n_cores]
        self.mesh = Mesh(np.asarray(devices), ("core",))
        in_specs = (PartitionSpec("core"),) * (n_params + len(out_names))
        out_specs = (PartitionSpec("core"),) * len(out_names)
        self.fn = jax.jit(shard_map(_body, mesh=self.mesh, in_specs=in_specs,
                                    out_specs=out_specs, check_rep=False),
                          keep_unused=True)
        self._shard = jax.sharding.NamedSharding(self.mesh, PartitionSpec("core"))

    def put_inputs(self, in_maps):
        arrs = []
        for name in self.in_names:
            cat = np.concatenate([np.asarray(m[name]) for m in in_maps], axis=0)
            arrs.append(jax.device_put(cat, self._shard))
        for av in self.out_avals:
            z = np.zeros((self.n_cores * av.shape[0], *av.shape[1:]), av.dtype)
            arrs.append(jax.device_put(z, self._shard))
        return arrs

    def run(self, dev_args):
        outs = self.fn(*dev_args)
        jax.block_until_ready(outs)
        return outs

    def results(self, outs):
        per_core = []
        for c in range(self.n_cores):
            per_core.append({
                name: np.asarray(outs[i]).reshape(
                    self.n_cores, *self.out_avals[i].shape)[c]
                for i, name in enumerate(self.out_names)})
        return per_core


# ---------------------------------------------------------------- host side

def _mask_blocks(mask):
    """Classify transposed 128x512 blocks of the [T,T] mask.

    Returns (block_info, uniq) where block_info[j][i] is None (all masked),
    -1 (all valid), or an index into uniq (mixed block patterns [128,512]).
    """
    m2 = np.asarray(mask).reshape(T, T)
    valid = (m2 != -np.inf)          # [tq, tk]
    validT = valid.T                 # [tk, tq]
    uniq, keys = [], {}
    block_info = []
    for j in range(NJ):
        row = []
        for i in range(NT):
            blk = validT[128 * i:128 * (i + 1), 512 * j:512 * (j + 1)]
            if not blk.any():
                row.append(None)
            elif blk.all():
                row.append(-1)
            else:
                k = hashlib.sha1(np.ascontiguousarray(blk)).hexdigest()
                if k not in keys:
                    keys[k] = len(uniq)
                    uniq.append(blk.astype(np.float32))
                row.append(keys[k])
        block_info.append(row)
    return block_info, uniq


_CACHE = {}


def _get_runner(block_info, n_uniq, loop_n=None, phases="BCD", cast_dma=True):
    key = (str(block_info), n_uniq, loop_n, phases, cast_dma)
    if key not in _CACHE:
        nc = _build_nc(block_info, n_uniq, loop_n=loop_n, phases=phases, cast_dma=cast_dma)
        _CACHE[key] = _SpmdRunner(nc, N_CORES)
    return _CACHE[key]


def _round_f32r(a):
    """Round fp32 to f32r (RNE to 11 mantissa bits) so device DMA loads are
    PE-ready without a casting pass."""
    x = np.ascontiguousarray(a, np.float32).view(np.uint32).astype(np.uint64)
    half = np.uint64(0x7FF)
    out = (x + half + ((x >> np.uint64(12)) & np.uint64(1))) & np.uint64(0xFFFFF000)
    return out.astype(np.uint32).view(np.float32)


def _pack_rows(a):
    """[R*128, F] -> [128, R*F]: partition-contiguous packing for fast DMA."""
    r = a.shape[0] // 128
    return np.ascontiguousarray(
        a.reshape(r, 128, a.shape[1]).transpose(1, 0, 2).reshape(128, -1))


def _make_in_maps(x, mask, wq, wk, wv, wo):
    block_info, uniq = _mask_blocks(mask)
    mk = (np.stack(uniq) if uniq
          else np.zeros((1, 128, 512), np.float32))
    # [u,128,512] -> [128, u*512]
    mk = np.ascontiguousarray(mk.transpose(1, 0, 2).reshape(128, -1))
    x = np.asarray(x, np.float32)
    in_maps = []
    for c in range(N_CORES):
        b, g = c // 4, c % 4
        sl = slice(HC * g, HC * (g + 1))
        in_maps.append({
            "xT": _round_f32r(x[b].T),
            "wqT": _pack_rows(_round_f32r(np.asarray(wq)[sl, :].T)),
            "wkT": _pack_rows(_round_f32r(np.asarray(wk)[sl, :].T)),
            "wvT": _pack_rows(_round_f32r(np.asarray(wv)[sl, :].T)),
            "woT": _pack_rows(_round_f32r(np.asarray(wo)[:, sl].T)),
            "mk": mk,
        })
    return in_maps, block_info, len(uniq)


def kernel(x, mask, wq, wk, wv, wo):
    in_maps, block_info, n_uniq = _make_in_maps(x, mask, wq, wk, wv, wo)
    runner = _get_runner(block_info, n_uniq)
    dev = runner.put_inputs(in_maps)
    res = runner.results(runner.run(dev))
    out = np.zeros((B, T, C), np.float32)
    for c in range(N_CORES):
        out[c // 4] += res[c]["y"]
    return out



# revision 11
# speedup vs baseline: 1.2307x; 1.2307x over previous
"""Causal multi-head attention (B=2, T=2048, C=1024, H=16, D=64) on 8 trn2 cores.

Sharding: core c -> (batch c//4, head-group c%4 of 4 heads / 256 channels).
Each core computes q/k/v for its head group, causal attention, and a partial
output projection y_part[2048,1024] = attnout_g @ wo_g.T. The host sums the 4
per-group partials of each batch (the "all-reduce after wo" done host-side).

Device kernel (per core, SPMD identical program):
  phase B: xT,[wq|wk]T loaded as float32r; q^T,k^T (head-channel major) via PE
           matmuls; v computed in NATURAL layout (x-tiles as stationary PE
           weights) straight into vaug (bf16, with a ones column).
  phase C: per head, per tq-tile(512): ST[tk128,tq512] = k^T.T @ q^T (K=128
           zero-padded), P = exp(ST/8) on ScalarE (PSUM->SBUF, bf16),
           diagonal blocks masked by 0/1 bf16 mask tiles on DVE (2x mode),
           PV accumulated over tk into PSUM[128,512] where row 64 (ones
           column of vaug) is the softmax denominator; normalize with
           reciprocal + K=1 broadcast matmul. The two outermost diagonal
           blocks only touch columns 256:512 (scores/exp/mask/PV narrowed).
  phase D: y[t,1024] = attnoutT.T @ woT in bf16, staged to SBUF (DVE/Pool
           split) and DMA'd out per 256-row tile as bf16 partials.

Engine budget per iteration (cost model): PE ~119us, ACT ~76us (exp),
DVE ~60us, Pool ~20us. Matmuls f32r for x/q/k path, bf16 for P/V/wo path.
"""
import time
import hashlib
import numpy as np
import ml_dtypes

import jax
import jax.numpy as jnp
from jax.sharding import Mesh, PartitionSpec
from jax.experimental.shard_map import shard_map

import concourse.bass as bass
import concourse.tile as tile
from concourse import bacc, mybir
from concourse import bass2jax
from concourse.bass2jax import _bass_exec_p, install_neuronx_cc_hook, partition_id_tensor

B, T, C = 2, 2048, 1024
H = 16
D = C // H            # 64
SCALE = D ** -0.5     # 0.125
N_CORES = 8
HG = H // (N_CORES // B)   # heads per core = 4
HC = HG * D                # channels per core = 256
KT = C // 128              # 8 contraction tiles
NT = T // 128              # 16 row tiles
NJ = T // 512              # 4 tq tiles
F32 = mybir.dt.float32
F32R = mybir.dt.float32r
BF16 = mybir.dt.bfloat16


# ---------------------------------------------------------------- device code

def _build_nc(block_info, n_uniq, loop_n=None, phases="BCD", cast_dma=False):
    """block_info[j][i] = None (skip) | -1 (full) | idx>=0 (mask tile index).

    x/q/k inputs are float32r (host pre-rounds to 11 mantissa bits); the
    P/V/attnout/wo path runs in bf16 (mask muls hit DVE 2x mode, wo DMA and
    y output halve).
    """
    nc = bacc.Bacc("TRN2", target_bir_lowering=False, debug=False,
                   num_devices=N_CORES)
    # x and w in bf16: halves the serial input-DMA time that gates phase C
    # start; PE rate is identical to f32r and precision stays well inside
    # the 2e-2 budget. q^T/k^T stay f32r on-chip for the score matmuls.
    xT_ap = nc.dram_tensor("xT", [C, T], BF16, kind="ExternalInput").ap()
    # weights host-packed so each SBUF partition's bytes are contiguous in
    # DRAM (8KB descriptors; scattered 1KB descriptors measured ~4x slower)
    wqT_ap = nc.dram_tensor("wqT", [128, KT * HC], BF16, kind="ExternalInput").ap()
    wkT_ap = nc.dram_tensor("wkT", [128, KT * HC], BF16, kind="ExternalInput").ap()
    wvT_ap = nc.dram_tensor("wvT", [128, KT * HC], BF16, kind="ExternalInput").ap()
    woT_ap = nc.dram_tensor("woT", [128, 2 * C], BF16, kind="ExternalInput").ap()
    mk_ap = nc.dram_tensor("mk", [128, max(n_uniq, 1) * 512], BF16,
                           kind="ExternalInput").ap()
    y_ap = nc.dram_tensor("y", [T, C], BF16, kind="ExternalOutput").ap()

    with tile.TileContext(nc) as tc:
        with (
            nc.allow_low_precision(reason="f32r/bf16 matmul pipeline"),
            tc.tile_pool(name="glob", bufs=1) as pg,
            tc.tile_pool(name="warm", bufs=1) as pwarm,
        ):
            # persistent across phases
            qT = pg.tile([128, 2, T], F32R)        # [o-part, o-tile, t]
            # k^T zero-padded per head: head h lives in partition rows
            # 64*(h%2)..+64 of kTz[:, h, :], other rows stay 0 so the score
            # matmul runs at K=128 (fp32r is slow for K<128).
            kTz = pg.tile([128, HG, T], F32R)
            # v natural per tk-tile/head, padded to 128 cols: [v | 1 | junk]
            # (ones column gives softmax denom; cols 65+ hit only pv rows 65+
            # which are never read).
            vaug = pg.tile([128, NT, HG, 128], BF16)
            ones128 = pg.tile([128, 128], F32R)    # all-ones lhsT for bcast
            recipz = pg.tile([128, 2, 512], F32R)  # row0=recip, rows1-127 zero

            # warm the Exp table while DMAs run; f32r rejects memset, so
            # zeros/ones are produced via rounding copies from f32 tiles.
            wtile = pwarm.tile([1, 16], F32)
            nc.vector.memset(wtile[:], 0.0)
            nc.scalar.activation(wtile[:], wtile[:],
                                 mybir.ActivationFunctionType.Exp)
            zt = pwarm.tile([128, 512], F32)
            nc.vector.memset(zt[:], 0.0)
            nc.vector.tensor_copy(kTz[:].rearrange("p h (j f) -> p h j f", f=512), zt[:, None, None, :].broadcast_to([128, HG, NJ, 512]))
            nc.vector.tensor_copy(recipz[:], zt[:, None, :].broadcast_to([128, 2, 512]))
            nc.gpsimd.memset(vaug[:], 0.0)
            nc.gpsimd.memset(vaug[:, :, :, D], 1.0)
            ot = pwarm.tile([128, 16], F32)
            nc.vector.memset(ot[:], 1.0)
            nc.vector.tensor_copy(ones128[:], ot[:, 0:1].broadcast_to([128, 128]))

            def body():
                # SBUF pools for the whole body coexist (no aliasing stalls);
                # only PSUM pools are scoped to phases (8 banks total).
                with (
                    tc.tile_pool(name="ab", bufs=1) as pab,
                    tc.tile_pool(name="wp", bufs=3) as pwp,
                    tc.tile_pool(name="cd", bufs=1) as pcd,
                    tc.tile_pool(name="pt", bufs=6) as ppt,
                    tc.tile_pool(name="small", bufs=4) as psm,
                    tc.tile_pool(name="ys", bufs=3) as pys,
                ):
                    xT = pab.tile([128, KT, T], BF16)
                    # weights first: the DMA device is serial, and the first
                    # projection matmuls need w before any xT chunk is useful
                    w_ts = []
                    for w_ap, qk in ((wqT_ap, 0), (wkT_ap, 1), (wvT_ap, 2)):
                        w_t = pwp.tile([128, KT, HC], BF16, tag="w", name=f"w{qk}")
                        nc.sync.dma_start(w_t[:], w_ap.rearrange("p (k m) -> p k m", k=KT))
                        w_ts.append(w_t)
                    # chunked so projections start as soon as chunk 0 lands
                    for kc in range(KT):
                        nc.sync.dma_start(
                            xT[:, kc, :],
                            xT_ap.rearrange("(k p) t -> k p t", p=128)[kc])
                    # C-phase constants: issued after xT (DMA device is
                    # serial; these are needed only once C starts) but into
                    # coexisting SBUF so they land during B.
                    if n_uniq > 0:
                        mks = pcd.tile([128, max(n_uniq, 1), 512], BF16)
                        nc.sync.dma_start(mks[:], mk_ap.rearrange("p (u f) -> p u f", f=512))
                    woT = pcd.tile([128, 2, C], BF16)
                    nc.sync.dma_start(woT[:], woT_ap.rearrange("p (k m) -> p k m", k=2))
                    attnoutT = pcd.tile([128, 2, T], BF16)
                    if "B" not in phases:
                        return
                    body_bcd(pab, pwp, pcd, ppt, psm, pys, xT, w_ts,
                             mks if n_uniq > 0 else None, woT, attnoutT)

            def body_bcd(pab, pwp, pcd, ppt, psm, pys, xT, w_ts, mks, woT,
                         attnoutT):
                with (
                    tc.tile_pool(name="psq", bufs=4, space="PSUM") as psq,
                    tc.tile_pool(name="psv", bufs=3, space="PSUM") as psv,
                ):

                    # q^T, k^T: [o, t] = w_g @ x^T ; kc-outer
                    for w_t, qk in ((w_ts[0], 0), (w_ts[1], 1)):
                        for m in range(2):
                            pss = [psq.tile([128, 512], F32, tag="qkps", name=f"qkps{qk}_{m}_{j}") for j in range(NJ)]
                            for kc in range(KT):
                                for j in range(NJ):
                                    nc.tensor.matmul(
                                        pss[j][:],
                                        w_t[:, kc, 128 * m:128 * (m + 1)],
                                        xT[:, kc, 512 * j:512 * (j + 1)],
                                        start=(kc == 0), stop=(kc == KT - 1))
                            for j in range(NJ):
                                sl = slice(512 * j, 512 * (j + 1))
                                if qk == 0:
                                    nc.scalar.copy(qT[:, m, sl], pss[j][:])
                                else:
                                    # scatter psum head-halves into kTz rows
                                    nc.vector.tensor_copy(kTz[0:64, 2 * m, sl],
                                                          pss[j][0:64, :])
                                    nc.vector.tensor_copy(kTz[64:128, 2 * m + 1, sl],
                                                          pss[j][64:128, :])
                    # v natural: x-tiles stationary, wv streams; psum->vaug
                    wv_t = w_ts[2]
                    for i in range(NT):
                        ps = psv.tile([128, HC], F32, tag="vtp", name=f"vt{i}")
                        for kc in range(KT):
                            nc.tensor.matmul(
                                ps[:], xT[:, kc, 128 * i:128 * (i + 1)],
                                wv_t[:, kc, :],
                                start=(kc == 0), stop=(kc == KT - 1))
                        # GPSIMD can't read PSUM on hw; DVE does the copy
                        nc.vector.tensor_copy(
                            vaug[:, i, :, 0:D],
                            ps[:].rearrange("p (h d) -> p h d", h=HG))

                # -------- phase C+D: attention, interleaved with out-proj ----
                if "C" not in phases and "D" not in phases:
                    return
                with (
                    tc.tile_pool(name="cd", bufs=1) as pcd,
                    tc.tile_pool(name="pt", bufs=4) as ppt,
                    tc.tile_pool(name="small", bufs=4) as psm,
                    tc.tile_pool(name="ys", bufs=3) as pys,
                    tc.tile_pool(name="psst", bufs=2, space="PSUM") as psst,
                    tc.tile_pool(name="pspv", bufs=2, space="PSUM") as pspv,
                    tc.tile_pool(name="psy", bufs=2, space="PSUM") as psy,
                ):
                    if n_uniq > 0:
                        mks = pcd.tile([128, max(n_uniq, 1), 512], BF16)
                        nc.sync.dma_start(mks[:], mk_ap.rearrange("p (u f) -> p u f", f=512))
                    woT = pcd.tile([128, 2, C], BF16)
                    nc.sync.dma_start(woT[:], woT_ap.rearrange("p (k m) -> p k m", k=2))
                    attnoutT = pcd.tile([128, 2, T], BF16)
                    norm_ctr = 0

                    for j in range(NJ):
                        # blocks: (tk-tile i, mask idx, narrow-start col)
                        blocks = []
                        for i, bi in enumerate(block_info[j]):
                            if bi is None:
                                continue
                            vstart = max(0, 128 * i - 512 * j)
                            blocks.append((i, bi, 256 if vstart >= 256 else 0))
                        chunks = [blocks[c:c + 2] for c in range(0, len(blocks), 2)]
                        for h in range(HG if "C" in phases else 0):
                            m = h // 2
                            jsl = slice(512 * j, 512 * (j + 1))
                            pv = pspv.tile([128, 512], F32, tag="pv", name=f"pv{h}_{j}")
                            n_acc = len(blocks)
                            acc = 0
                            prev_chunk = None  # (pt, [(i, s), ...])

                            def emit_pv(pt, idxs):
                                nonlocal acc
                                for c, (i, s) in enumerate(idxs):
                                    nc.tensor.matmul(
                                        pv[:, s:512], vaug[:, i, h, :],
                                        pt[:, c, s:512],
                                        start=(acc == 0), stop=(acc == n_acc - 1))
                                    acc += 1

                            for ch in chunks:
                                nsub = len(ch)
                                cs = min(s for _, _, s in ch)  # chunk col start
                                st = psst.tile([128, 2, 512], F32, tag="st", name=f"st{h}_{j}")
                                for c, (i, bi, s) in enumerate(ch):
                                    nc.tensor.matmul(
                                        st[:, c, s:512],
                                        kTz[:, h, 128 * i:128 * (i + 1)],
                                        qT[:, m, 512 * j + s:512 * (j + 1)],
                                        start=True, stop=True)
                                pt = ppt.tile([128, 2, 512], BF16, tag="pt")
                                # one exp per chunk: the fixed per-op ACT cost
                                # dominates, so amortize over up to 1024 cols
                                nc.scalar.activation(
                                    pt[:, 0:nsub, cs:512], st[:, 0:nsub, cs:512],
                                    mybir.ActivationFunctionType.Exp, scale=SCALE)
                                mi = [bi for _, bi, _ in ch]
                                if any(b >= 0 for b in mi):
                                    if nsub == 2 and mi[0] >= 0 and mi[1] == mi[0] + 1:
                                        nc.vector.tensor_mul(
                                            pt[:, :, cs:512], pt[:, :, cs:512],
                                            mks[:, mi[0]:mi[0] + 2, cs:512])
                                    else:
                                        for c, b in enumerate(mi):
                                            if b >= 0:
                                                nc.vector.tensor_mul(
                                                    pt[:, c, cs:512], pt[:, c, cs:512],
                                                    mks[:, b, cs:512])
                                if prev_chunk is not None:
                                    emit_pv(*prev_chunk)
                                prev_chunk = (pt, [(i, s) for i, _, s in ch])
                            emit_pv(*prev_chunk)
                            # normalization: recip of denom row, bcast via PE
                            slot = norm_ctr % 2
                            norm_ctr += 1
                            nc.vector.reciprocal(recipz[0:1, slot, :], pv[64:65, :])
                            bc = psy.tile([128, 512], F32, tag="yps", name=f"bc{h}_{j}")
                            nc.tensor.matmul(bc[:], ones128[:], recipz[:, slot, :],
                                             start=True, stop=True)
                            # engines may read only one PSUM operand: stage pv
                            avu = psm.tile([64, 512], F32, tag="avu")
                            nc.vector.tensor_copy(avu[:], pv[0:64, :])
                            row = 64 * (h % 2)
                            nc.vector.tensor_mul(
                                attnoutT[row:row + 64, m, jsl],
                                avu[:], bc[0:64, :])

                        # ---- phase D for this j: y rows [512j, 512j+512) ----
                        if "D" not in phases:
                            continue
                        for tp in range(2):     # pairs of row tiles
                            ys = pys.tile([128, 2, C], BF16, tag="ys")
                            for tsub in range(2):
                                t = 4 * j + 2 * tp + tsub
                                for o2 in range(2):
                                    ps = psy.tile([128, 512], F32, tag="yps", name=f"yps{t}_{o2}")
                                    for kc in range(2):
                                        nc.tensor.matmul(
                                            ps[:],
                                            attnoutT[:, kc, 128 * t:128 * (t + 1)],
                                            woT[:, kc, 512 * o2:512 * (o2 + 1)],
                                            start=(kc == 0), stop=(kc == 1))
                                    nc.vector.tensor_copy(
                                        ys[:, tsub, 512 * o2:512 * (o2 + 1)], ps[:])
                            r0 = 512 * j + 256 * tp
                            nc.gpsimd.dma_start(
                                y_ap[r0:r0 + 256, :].rearrange("(tt p) o -> p tt o", p=128),
                                ys[:])

            if loop_n is None:
                body()
            else:
                with tc.For_i(0, loop_n, 1):
                    body()

    nc.compile()
    return nc


# ---------------------------------------------------------------- run harness

def _install_verbose_hook():
    install_neuronx_cc_hook()
    try:
        import libneuronxla
    except ImportError:
        return
    import traceback
    inner = bass2jax.neuronx_cc_hook

    def wrapped(*a, **kw):
        try:
            return inner(*a, **kw)
        except BaseException:
            traceback.print_exc()
            raise
    libneuronxla.neuronx_cc = wrapped


class _SpmdRunner:
    def __init__(self, nc, n_cores):
        _install_verbose_hook()
        self.nc, self.n_cores = nc, n_cores
        pname = nc.partition_id_tensor.name if nc.partition_id_tensor else None
        in_names, out_names, out_avals = [], [], []
        for alloc in nc.m.functions[0].allocations:
            if not isinstance(alloc, mybir.MemoryLocationSet):
                continue
            name = alloc.memorylocations[0].name
            if alloc.kind == "ExternalInput":
                if name != pname:
                    in_names.append(name)
            elif alloc.kind == "ExternalOutput":
                out_names.append(name)
                out_avals.append(jax.core.ShapedArray(
                    tuple(alloc.tensor_shape), mybir.dt.np(alloc.dtype)))
        self.in_names, self.out_names, self.out_avals = in_names, out_names, out_avals
        n_params = len(in_names)
        all_in = list(in_names) + list(out_names)
        if pname is not None:
            all_in.append(pname)

        def _body(*args):
            operands = list(args)
            if pname is not None:
                operands.append(partition_id_tensor())
            return tuple(_bass_exec_p.bind(
                *operands,
                out_avals=tuple(out_avals), in_names=tuple(all_in),
                out_names=tuple(out_names), lowering_input_output_aliases=(),
                sim_require_finite=True, sim_require_nnan=True, nc=nc))

        devices = jax.devices()[:n_cores]
        self.mesh = Mesh(np.asarray(devices), ("core",))
        in_specs = (PartitionSpec("core"),) * (n_params + len(out_names))
        out_specs = (PartitionSpec("core"),) * len(out_names)
        self.fn = jax.jit(shard_map(_body, mesh=self.mesh, in_specs=in_specs,
                                    out_specs=out_specs, check_rep=False),
                          keep_unused=True)
        self._shard = jax.sharding.NamedSharding(self.mesh, PartitionSpec("core"))

    def put_inputs(self, in_maps):
        arrs = []
        for name in self.in_names:
            cat = np.concatenate([np.asarray(m[name]) for m in in_maps], axis=0)
            arrs.append(jax.device_put(cat, self._shard))
        for av in self.out_avals:
            z = np.zeros((self.n_cores * av.shape[0], *av.shape[1:]), av.dtype)
            arrs.append(jax.device_put(z, self._shard))
        return arrs

    def run(self, dev_args):
        outs = self.fn(*dev_args)
        jax.block_until_ready(outs)
        return outs

    def results(self, outs):
        per_core = []
        for c in range(self.n_cores):
            per_core.append({
                name: np.asarray(outs[i]).reshape(
                    self.n_cores, *self.out_avals[i].shape)[c]
                for i, name in enumerate(self.out_names)})
        return per_core


# ---------------------------------------------------------------- host side

def _mask_blocks(mask):
    """Classify transposed 128x512 blocks of the [T,T] mask.

    Returns (block_info, uniq) where block_info[j][i] is None (all masked),
    -1 (all valid), or an index into uniq (mixed block patterns [128,512]).
    """
    m2 = np.asarray(mask).reshape(T, T)
    valid = (m2 != -np.inf)          # [tq, tk]
    validT = valid.T                 # [tk, tq]
    uniq, keys = [], {}
    block_info = []
    for j in range(NJ):
        row = []
        for i in range(NT):
            blk = validT[128 * i:128 * (i + 1), 512 * j:512 * (j + 1)]
            if not blk.any():
                row.append(None)
            elif blk.all():
                row.append(-1)
            else:
                k = hashlib.sha1(np.ascontiguousarray(blk)).hexdigest()
                if k not in keys:
                    keys[k] = len(uniq)
                    uniq.append(blk.astype(np.float32))
                row.append(keys[k])
        block_info.append(row)
    return block_info, uniq


_CACHE = {}


def _get_runner(block_info, n_uniq, loop_n=None, phases="BCD", cast_dma=True):
    key = (str(block_info), n_uniq, loop_n, phases, cast_dma)
    if key not in _CACHE:
        nc = _build_nc(block_info, n_uniq, loop_n=loop_n, phases=phases, cast_dma=cast_dma)
        _CACHE[key] = _SpmdRunner(nc, N_CORES)
    return _CACHE[key]


def _round_f32r(a):
    """Round fp32 to f32r (RNE to 11 mantissa bits) so device DMA loads are
    PE-ready without a casting pass."""
    x = np.ascontiguousarray(a, np.float32).view(np.uint32).astype(np.uint64)
    half = np.uint64(0x7FF)
    out = (x + half + ((x >> np.uint64(12)) & np.uint64(1))) & np.uint64(0xFFFFF000)
    return out.astype(np.uint32).view(np.float32)


def _pack_rows(a):
    """[R*128, F] -> [128, R*F]: partition-contiguous packing for fast DMA."""
    r = a.shape[0] // 128
    return np.ascontiguousarray(
        a.reshape(r, 128, a.shape[1]).transpose(1, 0, 2).reshape(128, -1))


def _make_in_maps(x, mask, wq, wk, wv, wo):
    block_info, uniq = _mask_blocks(mask)
    mk = (np.stack(uniq) if uniq
          else np.zeros((1, 128, 512), np.float32))
    # [u,128,512] -> [128, u*512], bf16 0/1 mask tiles
    mk = np.ascontiguousarray(
        mk.transpose(1, 0, 2).reshape(128, -1)).astype(ml_dtypes.bfloat16)
    x = np.asarray(x, np.float32)
    bf = ml_dtypes.bfloat16
    in_maps = []
    for c in range(N_CORES):
        b, g = c // 4, c % 4
        sl = slice(HC * g, HC * (g + 1))
        in_maps.append({
            "xT": np.ascontiguousarray(x[b].T).astype(bf),
            "wqT": _pack_rows(np.asarray(wq, np.float32)[sl, :].T.astype(bf)),
            "wkT": _pack_rows(np.asarray(wk, np.float32)[sl, :].T.astype(bf)),
            "wvT": _pack_rows(np.asarray(wv, np.float32)[sl, :].T.astype(bf)),
            "woT": _pack_rows(np.asarray(wo, np.float32)[:, sl].T.astype(bf)),
            "mk": mk,
        })
    return in_maps, block_info, len(uniq)


def kernel(x, mask, wq, wk, wv, wo):
    in_maps, block_info, n_uniq = _make_in_maps(x, mask, wq, wk, wv, wo)
    runner = _get_runner(block_info, n_uniq)
    dev = runner.put_inputs(in_maps)
    res = runner.results(runner.run(dev))
    out = np.zeros((B, T, C), np.float32)
    for c in range(N_CORES):
        out[c // 4] += res[c]["y"].astype(np.float32)
    return out
